# revision 3
# baseline (speedup 1.0000x reference)
"""TRN2 Bass/Tile kernel: causal self-attention with RoPE.

Sharding across 8 NeuronCores: batch (2) x head-groups (4 groups of 4 heads,
tensor-parallel). Each core computes, for its batch and its 4 heads:
Q/K/V projections (RoPE folded into doubled Q/K weight matmuls), causal
softmax attention in transposed (scores^T) orientation with the softmax
denominator obtained via an extra ones-column in V, and a partial output
projection. The host sums the 4 partial outputs per batch.

All matmuls run in float32r (TF32-like, full-rate for free dim >= 256,
fp32 PSUM accumulation); measured end-to-end rel error ~3e-4.
"""
import numpy as np
import ml_dtypes
import concourse.bass as bass
from concourse import bacc
import concourse.mybir as mybir
import concourse.tile as tile
from concourse.bass_utils import run_bass_kernel_spmd

B, S, D = 2, 2048, 1024
H, DK = 16, 64
THETA = 10000.0
ST = 512              # q-band / projection s-tile width
NSC = S // 128        # 16 s-chunks of 128
f32 = mybir.dt.float32
f32r = mybir.dt.float32r
bf16 = mybir.dt.bfloat16
AF = mybir.ActivationFunctionType
Alu = mybir.AluOpType

# v_aug layout per s-chunk, per head pair: A head [v(64) | one],
# B head [one | zeros(63) | v(64)] (places attn rows at psum partitions 64:128)
VA = 65
VB = 128
VHP = VA + VB        # 193
VSC = 2 * VHP        # 386

_NC = None


def _build():
    nc = bacc.Bacc()
    xT = nc.dram_tensor("xT", [D, S], f32r, kind="ExternalInput")
    pw = nc.dram_tensor("pw", [D, 1280], f32r, kind="ExternalInput")
    woT = nc.dram_tensor("woT", [256, D], f32r, kind="ExternalInput")
    cossin = nc.dram_tensor("cossin", [128, 2 * S], f32, kind="ExternalInput")
    masks = nc.dram_tensor("masks", [128, 4096], bf16, kind="ExternalInput")
    vpat = nc.dram_tensor("vpat", [128, NSC * VSC], f32r, kind="ExternalInput")
    onesd = nc.dram_tensor("onesd", [128, 128], f32r, kind="ExternalInput")
    out = nc.dram_tensor("out", [S, D], f32, kind="ExternalOutput")

    with tile.TileContext(nc) as tc:
        with tc.tile_pool(name="persist", bufs=1) as pp:
            qT = [pp.tile([128, S], f32r, tag=f"qT{i}", name=f"qT{i}") for i in range(2)]
            kT = [pp.tile([128, S], f32r, tag=f"kT{i}", name=f"kT{i}") for i in range(2)]
            v_aug = pp.tile([128, NSC * VSC], f32r, tag="vaug")
            concatT = [pp.tile([128, S], f32r, tag=f"cT{i}", name=f"cT{i}") for i in range(2)]
            woT_sb = pp.tile([128, 2, D], f32r, tag="woT")
            ones_sb = pp.tile([128, 128], f32r, tag="ones")
            masks_sb = pp.tile([128, 4096], bf16, tag="masks")

            nc.sync.dma_start(masks_sb[:], masks[:])
            nc.sync.dma_start(ones_sb[:], onesd[:])
            nc.sync.dma_start(v_aug[:], vpat[:])
            nc.sync.dma_start(woT_sb[:],
                              woT[:].rearrange("(k p) m -> p k m", p=128))

            va_sc = v_aug[:].rearrange("p (c h r) -> p c h r", c=NSC, r=VHP)

            # ---- Phase A: projections + RoPE + V ----
            with tc.tile_pool(name="pa", bufs=1) as pa, \
                 tc.tile_pool(name="pax", bufs=2) as pax, \
                 tc.tile_pool(name="prope", bufs=3) as prope, \
                 tc.tile_pool(name="psA", bufs=5, space="PSUM") as psA, \
                 tc.tile_pool(name="psV", bufs=2, space="PSUM") as psV:
                pw_sb = pa.tile([128, 8, 1280], f32r, tag="pw")
                cs_sb = pa.tile([128, 2, S], f32, tag="cs")
                nc.sync.dma_start(pw_sb[:],
                                  pw[:].rearrange("(k p) m -> p k m", p=128))
                nc.sync.dma_start(cs_sb[:],
                                  cossin[:].rearrange("p (c s) -> p c s", c=2))

                for st in range(4):
                    xs = pax.tile([128, 8, ST], f32r, tag="xs")
                    nc.sync.dma_start(
                        xs[:],
                        xT[:, st * ST:(st + 1) * ST]
                        .rearrange("(k p) m -> p k m", p=128))
                    sl = slice(st * ST, (st + 1) * ST)
                    for hp in range(2):
                        for aoff, boff, dst in ((0, 256, qT), (512, 768, kT)):
                            pa_ps = psA.tile([128, ST], f32, tag="proj")
                            pb_ps = psA.tile([128, ST], f32, tag="proj")
                            ao = aoff + 128 * hp
                            bo = boff + 128 * hp
                            for kt in range(8):
                                nc.tensor.matmul(pa_ps[:],
                                                 pw_sb[:, kt, ao:ao + 128],
                                                 xs[:, kt, :],
                                                 start=(kt == 0), stop=(kt == 7))
                            for kt in range(8):
                                nc.tensor.matmul(pb_ps[:],
                                                 pw_sb[:, kt, bo:bo + 128],
                                                 xs[:, kt, :],
                                                 start=(kt == 0), stop=(kt == 7))
                            t1 = prope.tile([128, ST], f32r, tag="ropea")
                            t2 = prope.tile([128, ST], f32r, tag="ropeb")
                            nc.vector.tensor_tensor(t1[:], pa_ps[:],
                                                    cs_sb[:, 0, sl], Alu.mult)
                            nc.vector.tensor_tensor(t2[:], pb_ps[:],
                                                    cs_sb[:, 1, sl], Alu.mult)
                            nc.vector.tensor_tensor(dst[hp][:, sl], t1[:],
                                                    t2[:], Alu.add)
                    for scl in range(4):
                        sc = st * 4 + scl
                        vp = psV.tile([128, 256], f32, tag="vproj")
                        for kt in range(8):
                            nc.tensor.matmul(vp[:],
                                             xs[:, kt, scl * 128:(scl + 1) * 128],
                                             pw_sb[:, kt, 1024:1280],
                                             start=(kt == 0), stop=(kt == 7))
                        vp_r = vp[:].rearrange("p (g t e) -> p g t e", g=2, t=2)
                        nc.vector.tensor_copy(va_sc[:, sc, :, 0:64],
                                              vp_r[:, :, 0, :])
                        nc.vector.tensor_copy(va_sc[:, sc, :, VA + 64:VHP],
                                              vp_r[:, :, 1, :])

            # ---- Phase B: attention (scores^T -> exp -> PV -> normalize) ----
            with tc.tile_pool(name="wtp", bufs=3) as wtp, \
                 tc.tile_pool(name="dnp", bufs=2) as dnp, \
                 tc.tile_pool(name="bcp", bufs=2) as bcp, \
                 tc.tile_pool(name="psS", bufs=2, space="PSUM") as psS, \
                 tc.tile_pool(name="psP", bufs=2, space="PSUM") as psP, \
                 tc.tile_pool(name="psB", bufs=1, space="PSUM") as psB:
                for band in range(4):
                    qsl = slice(band * ST, (band + 1) * ST)
                    nkt = 4 * band + 4
                    for hp in range(2):
                        pvA = psP.tile([65, ST], f32, tag="pv")
                        pvB = psP.tile([128, ST], f32, tag="pv")
                        for kt in range(nkt):
                            ksl = slice(kt * 128, (kt + 1) * 128)
                            scp = psS.tile([128, 1024], f32, tag="sc")
                            nc.tensor.matmul(scp[:, 0:512],
                                             kT[hp][0:64, ksl],
                                             qT[hp][0:64, qsl],
                                             start=True, stop=True)
                            nc.tensor.matmul(scp[:, 512:1024],
                                             kT[hp][64:128, ksl],
                                             qT[hp][64:128, qsl],
                                             start=True, stop=True)
                            wt = wtp.tile([128, 1024], f32r, tag="wt")
                            nc.scalar.activation(wt[:], scp[:], AF.Exp,
                                                 scale=0.125)
                            j = kt - 4 * band
                            if j >= 0:
                                nc.gpsimd.tensor_tensor(
                                    wt[:], wt[:],
                                    masks_sb[:, j * 1024:(j + 1) * 1024],
                                    Alu.mult)
                            nc.tensor.matmul(pvA[:],
                                             va_sc[:, kt, hp, 0:VA],
                                             wt[:, 0:512],
                                             start=(kt == 0),
                                             stop=(kt == nkt - 1),
                                             skip_group_check=True)
                            nc.tensor.matmul(pvB[:],
                                             va_sc[:, kt, hp, VA:VHP],
                                             wt[:, 512:1024],
                                             start=(kt == 0),
                                             stop=(kt == nkt - 1),
                                             skip_group_check=True)
                        # normalize head A (denominator at pvA row 64)
                        dnA = dnp.tile([65, ST], f32r, tag="dna")
                        nc.scalar.copy(dnA[64:65, :], pvA[64:65, :])
                        bcA_ps = psB.tile([64, ST], f32, tag="bca")
                        nc.tensor.matmul(bcA_ps[:], ones_sb[64:65, 0:64],
                                         dnA[64:65, :], start=True, stop=True)
                        bcA = bcp.tile([64, ST], f32, tag="bca")
                        nc.vector.reciprocal_approx_fast(bcA[:], bcA_ps[:])
                        nc.vector.tensor_tensor(concatT[hp][0:64, qsl],
                                                pvA[0:64, :], bcA[:], Alu.mult)
                        # normalize head B (denominator at pvB row 0,
                        # attn rows at 64:128)
                        rB = dnp.tile([65, ST], f32, tag="rb")
                        nc.vector.reciprocal_approx_fast(rB[0:1, :],
                                                         pvB[0:1, :])
                        dnB = dnp.tile([65, ST], f32r, tag="dnb")
                        nc.scalar.copy(dnB[0:1, :], rB[0:1, :])
                        bcB_ps = psB.tile([128, ST], f32, tag="bcb")
                        nc.tensor.matmul(bcB_ps[:], ones_sb[0:1, :],
                                         dnB[0:1, :], start=True, stop=True)
                        bcB = bcp.tile([128, ST], f32, tag="bcb")
                        nc.scalar.copy(bcB[64:128, :], bcB_ps[64:128, :])
                        nc.vector.tensor_tensor(concatT[hp][64:128, qsl],
                                                pvB[64:128, :], bcB[64:128, :],
                                                Alu.mult)

            # ---- Phase C: output projection (partial) ----
            with tc.tile_pool(name="outp", bufs=3) as outp, \
                 tc.tile_pool(name="psO", bufs=2, space="PSUM") as psO:
                for sc in range(NSC):
                    ssl = slice(sc * 128, (sc + 1) * 128)
                    op_ps = psO.tile([128, D], f32, tag="op")
                    for ds in range(2):
                        dsl = slice(ds * 512, (ds + 1) * 512)
                        for ot in range(2):
                            nc.tensor.matmul(op_ps[:, dsl],
                                             concatT[ot][:, ssl],
                                             woT_sb[:, ot, dsl],
                                             start=(ot == 0), stop=(ot == 1))
                    ob = outp.tile([128, D], f32, tag="ob")
                    if sc % 2 == 0:
                        nc.scalar.copy(ob[:], op_ps[:])
                    else:
                        nc.vector.tensor_copy(ob[:], op_ps[:])
                    nc.sync.dma_start(out[ssl, :], ob[:])
    nc.finalize()
    return nc


def _rope_tables():
    inv_freq = 1.0 / (THETA ** (np.arange(0, DK, 2, dtype=np.float64) / DK))
    t = np.arange(S, dtype=np.float64)
    freqs = np.outer(t, inv_freq)
    emb = np.stack((freqs, freqs), axis=-1).reshape(S, DK)
    return np.cos(emb).astype(np.float32), np.sin(emb).astype(np.float32)


def _sgn_shuf(w):
    ws = np.empty_like(w)
    ws[0::2] = -w[1::2]
    ws[1::2] = w[0::2]
    return ws


def _host_consts():
    f_idx = np.arange(512)
    p_idx = np.arange(128)
    mblocks = []
    for j in range(4):
        mj = (f_idx[None, :] >= p_idx[:, None] + 128 * j).astype(np.float32)
        mblocks.append(np.tile(mj, (1, 2)))
    masks_np = np.concatenate(mblocks, axis=1).astype(ml_dtypes.bfloat16)

    vpat_np = np.zeros((128, NSC * VSC), np.float32)
    for sc in range(NSC):
        for r in range(2):
            base = sc * VSC + r * VHP
            vpat_np[:, base + 64] = 1.0   # A ones column
            vpat_np[:, base + VA] = 1.0   # B ones column

    onesd_np = np.zeros((128, 128), np.float32)
    onesd_np[64, 0:64] = 1.0              # lhsT for head-A broadcast
    onesd_np[0, 64:128] = 1.0             # lhsT for head-B broadcast
    return masks_np, vpat_np, onesd_np


def kernel(x, token_positions, W_q, W_k, W_v, W_o):
    global _NC
    if _NC is None:
        _NC = _build()
    x = np.asarray(x, dtype=np.float32)
    token_positions = np.asarray(token_positions)
    W_q = np.asarray(W_q, dtype=np.float32)
    W_k = np.asarray(W_k, dtype=np.float32)
    W_v = np.asarray(W_v, dtype=np.float32)
    W_o = np.asarray(W_o, dtype=np.float32)

    cos_t, sin_t = _rope_tables()
    masks_np, vpat_np, onesd_np = _host_consts()

    in_maps = []
    for c in range(8):
        b, g = divmod(c, 4)
        rows = slice(256 * g, 256 * (g + 1))
        wq, wk, wv = W_q[rows], W_k[rows], W_v[rows]
        pw_np = np.ascontiguousarray(np.concatenate(
            [wq.T, _sgn_shuf(wq).T, wk.T, _sgn_shuf(wk).T, wv.T], axis=1))
        woT_np = np.ascontiguousarray(W_o[:, rows].T)
        pos = np.asarray(token_positions[b], dtype=np.int64)
        cosT = np.tile(cos_t[pos].T, (2, 1))
        sinT = np.tile(sin_t[pos].T, (2, 1))
        cossin_np = np.ascontiguousarray(
            np.concatenate([cosT, sinT], axis=1), dtype=np.float32)
        xT_np = np.ascontiguousarray(x[b].T)
        in_maps.append({
            "xT": xT_np, "pw": pw_np, "woT": woT_np, "cossin": cossin_np,
            "masks": masks_np, "vpat": vpat_np, "onesd": onesd_np,
        })

    res = run_bass_kernel_spmd(_NC, in_maps, core_ids=list(range(8)))
    outs = [res.results[c]["out"] for c in range(8)]
    o0 = outs[0] + outs[1] + outs[2] + outs[3]
    o1 = outs[4] + outs[5] + outs[6] + outs[7]
    return np.stack([o0, o1]).astype(np.float32)


# revision 4
# speedup vs baseline: 1.0737x; 1.0737x over previous
"""TRN2 Bass/Tile kernel: causal self-attention with RoPE.

Sharding across 8 NeuronCores: batch (2) x head-groups (4 groups of 4 heads,
tensor-parallel). Each core computes, for its batch and its 4 heads:
Q/K/V projections (RoPE folded into doubled Q/K weight matmuls), causal
softmax attention in transposed (scores^T) orientation with the softmax
denominator obtained via an extra ones-column in V, and a partial output
projection. The host sums the 4 partial outputs per batch.

All matmuls run in float32r (TF32-like, full-rate for free dim >= 256,
fp32 PSUM accumulation); measured end-to-end rel error ~3e-4.
"""
import numpy as np
import ml_dtypes
import concourse.bass as bass
from concourse import bacc
import concourse.mybir as mybir
import concourse.tile as tile
from concourse.bass_utils import run_bass_kernel_spmd

B, S, D = 2, 2048, 1024
H, DK = 16, 64
THETA = 10000.0
ST = 512              # q-band / projection s-tile width
NSC = S // 128        # 16 s-chunks of 128
f32 = mybir.dt.float32
f32r = mybir.dt.float32r
bf16 = mybir.dt.bfloat16
AF = mybir.ActivationFunctionType
Alu = mybir.AluOpType

# v_aug layout per s-chunk, per head pair: A head [v(64) | one],
# B head [one | zeros(63) | v(64)] (places attn rows at psum partitions 64:128)
VA = 65
VB = 128
VHP = VA + VB        # 193
VSC = 2 * VHP        # 386

_NC = None


def _build():
    nc = bacc.Bacc()
    xT = nc.dram_tensor("xT", [D, S], f32r, kind="ExternalInput")
    pw = nc.dram_tensor("pw", [D, 1280], f32r, kind="ExternalInput")
    woT = nc.dram_tensor("woT", [256, D], f32r, kind="ExternalInput")
    cossin = nc.dram_tensor("cossin", [128, 2 * S], f32, kind="ExternalInput")
    masks = nc.dram_tensor("masks", [128, 4096], bf16, kind="ExternalInput")
    vpat = nc.dram_tensor("vpat", [128, NSC * VSC], f32r, kind="ExternalInput")
    onesd = nc.dram_tensor("onesd", [128, 128], f32r, kind="ExternalInput")
    out = nc.dram_tensor("out", [S, D], f32, kind="ExternalOutput")

    with tile.TileContext(nc) as tc:
        with tc.tile_pool(name="persist", bufs=1) as pp:
            qT = [pp.tile([128, S], f32r, tag=f"qT{i}", name=f"qT{i}") for i in range(2)]
            kT = [pp.tile([128, S], f32r, tag=f"kT{i}", name=f"kT{i}") for i in range(2)]
            v_aug = pp.tile([128, NSC * VSC], f32r, tag="vaug")
            concatT = [pp.tile([128, S], f32r, tag=f"cT{i}", name=f"cT{i}") for i in range(2)]
            woT_sb = pp.tile([128, 2, D], f32r, tag="woT")
            ones_sb = pp.tile([128, 128], f32r, tag="ones")
            masks_sb = pp.tile([128, 4096], bf16, tag="masks")

            nc.sync.dma_start(masks_sb[:], masks[:])
            nc.sync.dma_start(ones_sb[:], onesd[:])
            nc.sync.dma_start(v_aug[:], vpat[:])
            nc.sync.dma_start(woT_sb[:],
                              woT[:].rearrange("(k p) m -> p k m", p=128))

            va_sc = v_aug[:].rearrange("p (c h r) -> p c h r", c=NSC, r=VHP)

            # ---- Phase A: projections + RoPE + V ----
            with tc.tile_pool(name="pa", bufs=1) as pa, \
                 tc.tile_pool(name="pax", bufs=2) as pax, \
                 tc.tile_pool(name="prope", bufs=3) as prope, \
                 tc.tile_pool(name="psA", bufs=5, space="PSUM") as psA, \
                 tc.tile_pool(name="psV", bufs=2, space="PSUM") as psV:
                pw_sb = pa.tile([128, 8, 1280], f32r, tag="pw")
                cs_sb = pa.tile([128, 2, S], f32, tag="cs")
                nc.sync.dma_start(pw_sb[:],
                                  pw[:].rearrange("(k p) m -> p k m", p=128))
                nc.sync.dma_start(cs_sb[:],
                                  cossin[:].rearrange("p (c s) -> p c s", c=2))

                for st in range(4):
                    xs = pax.tile([128, 8, ST], f32r, tag="xs")
                    nc.sync.dma_start(
                        xs[:],
                        xT[:, st * ST:(st + 1) * ST]
                        .rearrange("(k p) m -> p k m", p=128))
                    sl = slice(st * ST, (st + 1) * ST)
                    for hp in range(2):
                        for aoff, boff, dst in ((0, 256, qT), (512, 768, kT)):
                            pa_ps = psA.tile([128, ST], f32, tag="proj")
                            pb_ps = psA.tile([128, ST], f32, tag="proj")
                            ao = aoff + 128 * hp
                            bo = boff + 128 * hp
                            for kt in range(8):
                                nc.tensor.matmul(pa_ps[:],
                                                 pw_sb[:, kt, ao:ao + 128],
                                                 xs[:, kt, :],
                                                 start=(kt == 0), stop=(kt == 7))
                            for kt in range(8):
                                nc.tensor.matmul(pb_ps[:],
                                                 pw_sb[:, kt, bo:bo + 128],
                                                 xs[:, kt, :],
                                                 start=(kt == 0), stop=(kt == 7))
                            t1 = prope.tile([128, ST], f32r, tag="ropea")
                            t2 = prope.tile([128, ST], f32r, tag="ropeb")
                            nc.vector.tensor_tensor(t1[:], pa_ps[:],
                                                    cs_sb[:, 0, sl], Alu.mult)
                            nc.vector.tensor_tensor(t2[:], pb_ps[:],
                                                    cs_sb[:, 1, sl], Alu.mult)
                            nc.vector.tensor_tensor(dst[hp][:, sl], t1[:],
                                                    t2[:], Alu.add)
                    for scl in range(4):
                        sc = st * 4 + scl
                        vp = psV.tile([128, 256], f32, tag="vproj")
                        for kt in range(8):
                            nc.tensor.matmul(vp[:],
                                             xs[:, kt, scl * 128:(scl + 1) * 128],
                                             pw_sb[:, kt, 1024:1280],
                                             start=(kt == 0), stop=(kt == 7))
                        vp_r = vp[:].rearrange("p (g t e) -> p g t e", g=2, t=2)
                        nc.vector.tensor_copy(va_sc[:, sc, :, 0:64],
                                              vp_r[:, :, 0, :])
                        nc.vector.tensor_copy(va_sc[:, sc, :, VA + 64:VHP],
                                              vp_r[:, :, 1, :])

            # ---- Phase B: attention (scores^T -> exp -> PV -> normalize) ----
            with tc.tile_pool(name="wtp", bufs=5) as wtp, \
                 tc.tile_pool(name="dnp", bufs=2) as dnp, \
                 tc.tile_pool(name="bcp", bufs=2) as bcp, \
                 tc.tile_pool(name="psS", bufs=2, space="PSUM") as psS, \
                 tc.tile_pool(name="psP", bufs=2, space="PSUM") as psP, \
                 tc.tile_pool(name="psB", bufs=1, space="PSUM") as psB:
                for band in range(4):
                    qsl = slice(band * ST, (band + 1) * ST)
                    nkt = 4 * band + 4
                    # diagonal k-tiles first: their mask multiply runs on
                    # gpsimd while PE/ACT stream the full (unmasked) k-tiles
                    kts = list(range(4 * band, nkt)) + list(range(0, 4 * band))
                    for hp in range(2):
                        pvA = psP.tile([65, ST], f32, tag="pv")
                        pvB = psP.tile([128, ST], f32, tag="pv")
                        for i, kt in enumerate(kts):
                            ksl = slice(kt * 128, (kt + 1) * 128)
                            scp = psS.tile([128, 1024], f32, tag="sc")
                            nc.tensor.matmul(scp[:, 0:512],
                                             kT[hp][0:64, ksl],
                                             qT[hp][0:64, qsl],
                                             start=True, stop=True)
                            nc.tensor.matmul(scp[:, 512:1024],
                                             kT[hp][64:128, ksl],
                                             qT[hp][64:128, qsl],
                                             start=True, stop=True)
                            wt = wtp.tile([128, 1024], f32r, tag="wt")
                            nc.scalar.activation(wt[:], scp[:], AF.Exp,
                                                 scale=0.125)
                            j = kt - 4 * band
                            if j >= 0:
                                eng = nc.vector if band == 0 else nc.gpsimd
                                eng.tensor_tensor(
                                    wt[:], wt[:],
                                    masks_sb[:, j * 1024:(j + 1) * 1024],
                                    Alu.mult)
                            nc.tensor.matmul(pvA[:],
                                             va_sc[:, kt, hp, 0:VA],
                                             wt[:, 0:512],
                                             start=(i == 0),
                                             stop=(i == nkt - 1),
                                             skip_group_check=True)
                            nc.tensor.matmul(pvB[:],
                                             va_sc[:, kt, hp, VA:VHP],
                                             wt[:, 512:1024],
                                             start=(i == 0),
                                             stop=(i == nkt - 1),
                                             skip_group_check=True)
                        # normalize head A (denominator at pvA row 64)
                        dnA = dnp.tile([65, ST], f32r, tag="dna")
                        nc.scalar.copy(dnA[64:65, :], pvA[64:65, :])
                        bcA_ps = psB.tile([64, ST], f32, tag="bca")
                        nc.tensor.matmul(bcA_ps[:], ones_sb[64:65, 0:64],
                                         dnA[64:65, :], start=True, stop=True)
                        bcA = bcp.tile([64, ST], f32, tag="bca")
                        nc.vector.reciprocal_approx_fast(bcA[:], bcA_ps[:])
                        nc.vector.tensor_tensor(concatT[hp][0:64, qsl],
                                                pvA[0:64, :], bcA[:], Alu.mult)
                        # normalize head B (denominator at pvB row 0,
                        # attn rows at 64:128)
                        rB = dnp.tile([65, ST], f32, tag="rb")
                        nc.vector.reciprocal_approx_fast(rB[0:1, :],
                                                         pvB[0:1, :])
                        dnB = dnp.tile([65, ST], f32r, tag="dnb")
                        nc.scalar.copy(dnB[0:1, :], rB[0:1, :])
                        bcB_ps = psB.tile([128, ST], f32, tag="bcb")
                        nc.tensor.matmul(bcB_ps[:], ones_sb[0:1, :],
                                         dnB[0:1, :], start=True, stop=True)
                        bcB = bcp.tile([128, ST], f32, tag="bcb")
                        nc.scalar.copy(bcB[64:128, :], bcB_ps[64:128, :])
                        nc.vector.tensor_tensor(concatT[hp][64:128, qsl],
                                                pvB[64:128, :], bcB[64:128, :],
                                                Alu.mult)

            # ---- Phase C: output projection (partial) ----
            with tc.tile_pool(name="outp", bufs=3) as outp, \
                 tc.tile_pool(name="psO", bufs=2, space="PSUM") as psO:
                for sc in range(NSC):
                    ssl = slice(sc * 128, (sc + 1) * 128)
                    op_ps = psO.tile([128, D], f32, tag="op")
                    for ds in range(2):
                        dsl = slice(ds * 512, (ds + 1) * 512)
                        for ot in range(2):
                            nc.tensor.matmul(op_ps[:, dsl],
                                             concatT[ot][:, ssl],
                                             woT_sb[:, ot, dsl],
                                             start=(ot == 0), stop=(ot == 1))
                    ob = outp.tile([128, D], f32, tag="ob")
                    if sc % 2 == 0:
                        nc.scalar.copy(ob[:], op_ps[:])
                    else:
                        nc.vector.tensor_copy(ob[:], op_ps[:])
                    nc.sync.dma_start(out[ssl, :], ob[:])
    nc.finalize()
    return nc


def _rope_tables():
    inv_freq = 1.0 / (THETA ** (np.arange(0, DK, 2, dtype=np.float64) / DK))
    t = np.arange(S, dtype=np.float64)
    freqs = np.outer(t, inv_freq)
    emb = np.stack((freqs, freqs), axis=-1).reshape(S, DK)
    return np.cos(emb).astype(np.float32), np.sin(emb).astype(np.float32)


def _sgn_shuf(w):
    ws = np.empty_like(w)
    ws[0::2] = -w[1::2]
    ws[1::2] = w[0::2]
    return ws


def _host_consts():
    f_idx = np.arange(512)
    p_idx = np.arange(128)
    mblocks = []
    for j in range(4):
        mj = (f_idx[None, :] >= p_idx[:, None] + 128 * j).astype(np.float32)
        mblocks.append(np.tile(mj, (1, 2)))
    masks_np = np.concatenate(mblocks, axis=1).astype(ml_dtypes.bfloat16)

    vpat_np = np.zeros((128, NSC * VSC), np.float32)
    for sc in range(NSC):
        for r in range(2):
            base = sc * VSC + r * VHP
            vpat_np[:, base + 64] = 1.0   # A ones column
            vpat_np[:, base + VA] = 1.0   # B ones column

    onesd_np = np.zeros((128, 128), np.float32)
    onesd_np[64, 0:64] = 1.0              # lhsT for head-A broadcast
    onesd_np[0, 64:128] = 1.0             # lhsT for head-B broadcast
    return masks_np, vpat_np, onesd_np


def kernel(x, token_positions, W_q, W_k, W_v, W_o):
    global _NC
    if _NC is None:
        _NC = _build()
    x = np.asarray(x, dtype=np.float32)
    token_positions = np.asarray(token_positions)
    W_q = np.asarray(W_q, dtype=np.float32)
    W_k = np.asarray(W_k, dtype=np.float32)
    W_v = np.asarray(W_v, dtype=np.float32)
    W_o = np.asarray(W_o, dtype=np.float32)

    cos_t, sin_t = _rope_tables()
    masks_np, vpat_np, onesd_np = _host_consts()

    in_maps = []
    for c in range(8):
        b, g = divmod(c, 4)
        rows = slice(256 * g, 256 * (g + 1))
        wq, wk, wv = W_q[rows], W_k[rows], W_v[rows]
        pw_np = np.ascontiguousarray(np.concatenate(
            [wq.T, _sgn_shuf(wq).T, wk.T, _sgn_shuf(wk).T, wv.T], axis=1))
        woT_np = np.ascontiguousarray(W_o[:, rows].T)
        pos = np.asarray(token_positions[b], dtype=np.int64)
        cosT = np.tile(cos_t[pos].T, (2, 1))
        sinT = np.tile(sin_t[pos].T, (2, 1))
        cossin_np = np.ascontiguousarray(
            np.concatenate([cosT, sinT], axis=1), dtype=np.float32)
        xT_np = np.ascontiguousarray(x[b].T)
        in_maps.append({
            "xT": xT_np, "pw": pw_np, "woT": woT_np, "cossin": cossin_np,
            "masks": masks_np, "vpat": vpat_np, "onesd": onesd_np,
        })

    res = run_bass_kernel_spmd(_NC, in_maps, core_ids=list(range(8)))
    outs = [res.results[c]["out"] for c in range(8)]
    o0 = outs[0] + outs[1] + outs[2] + outs[3]
    o1 = outs[4] + outs[5] + outs[6] + outs[7]
    return np.stack([o0, o1]).astype(np.float32)
